# revision 1
# baseline (speedup 1.0000x reference)
"""Dual-branch multi-head attention on 8 Trainium2 NeuronCores.

Problem (B, S, D, H, DH) = (4, 1024, 1024, 16, 64):
    q/k/v + sq/sk/sv projections of x, two softmax attentions, weighted sum.

Sharding: tensor-parallel over heads — core c owns heads {2c, 2c+1} of both
branches (output columns 128c..128c+128). Each core reads the full x
(pre-transposed on host to xT [D, B*S]) and its [D, 128] weight slices.
No collectives: host concatenates per-core outputs along the feature axis.

Per-core pipeline:
  q/k/sq/sk proj (transposed layouts, features on partitions):
          qT = Wq^T @ xT  (PSUM accum over 8 k-chunks, bias fused in the
          PSUM->SBUF copy). Scale 1/sqrt(DH) folded into Wq on host.
  v/sv proj (natural layout, tokens on partitions): per 128-token block,
          v_aug[token, col] = sum_kc xT_chunk.T @ vw_chunk  accumulated in
          PSUM, one K=1 matmul with a host-side ones row adds biases and
          writes the ones columns (softmax-denominator trick). vw packs
          [v_h0|1|v_h1|1|sv_h0|1|sv_h1|1] = 260 cols; combine weights
          softmax(attn_w) folded into Wv/Wsv on host. This kills the
          PE transposes + DVE copies a transposed v projection would need.
  scores: scoresT[j,i] = k^T.T @ qT with contraction DH=64; the two heads
          run as concurrent row-tiled matmuls (array rows 0-63 / 64-127).
  exp:    ACT PSUM->SBUF, no max subtraction (scores ~ N(0,1), exp safe).
  PV:     ctx_augT[65, i] = v_aug.T @ probsT accumulated over j-chunks;
          row 64 = softmax denominator (ones column).
  norm:   DVE reciprocal of denominator row + partition-broadcast multiply.
  out:    combine branches, PE-transpose back to [token, dh], DMA out.

Matmul dtype float32r (TF32-like, 4x faster than fp32 on trn2 PE);
producers write fp32r tiles directly (HW rounds on write).

All per-core inputs (xT, W slices, biases, identity, the packed v-weights)
live in ONE dram tensor `ipack`: per-buffer-handle dispatch cost through the
axon tunnel is ~0.5 ms/arg, so 15 args -> 2 (ipack + out) saves ~7 ms/call.
"""

import os
import numpy as np

import concourse.bass as bass
import concourse.bacc as bacc
import concourse.tile as tile
from concourse import mybir
from concourse.bass_utils import run_bass_kernel_spmd

dt = mybir.dt
Alu = mybir.AluOpType
Act = mybir.ActivationFunctionType

B, S, D, H, DH = 4, 1024, 1024, 16, 64
NCORES = 8
HPC = H // NCORES            # heads per core = 2
CW = HPC * DH                # output cols per core = 128
KC = D // 128                # contraction chunks = 8
JC = S // 128                # key-token chunks = 8
NIC = S // 512               # query i-chunks of 512 = 2
NTB = (B * S) // B // 512    # token blocks per batch = 2
VW = 2 * HPC * (DH + 1)      # packed v/sv moving-operand cols = 260

MM = {"f32r": dt.float32r, "f32": dt.float32}[os.environ.get("KMM", "f32r")]

PROJ4 = ["q", "k", "sq", "sk"]

# packed-input column offsets
XOFF = 0                      # xT [D, B*S]
WOFF = B * S                  # 4 weight slices [D, CW] (q, k, sq, sk)
BOFF = WOFF + 4 * CW          # 4 bias columns (rows 0:CW)
IOFF = BOFF + 4               # identity [128, DH]
VOFF = IOFF + DH              # vw [D, VW]
VBOFF = VOFF + VW             # vw bias row (row 0 only) [1, VW]
OOFF = VBOFF + VW             # ones row (row 0 only) [1, 128]
IPACK_COLS = OOFF + 128


def _emit(nc, tc, ctx, prm):
    """Emit the whole kernel under TileContext tc. prm: dram param handles."""
    f32 = dt.float32

    const = ctx.enter_context(tc.tile_pool(name="const", bufs=1))
    # bufs=17 keeps two batches' x chunks live so batch b+1's DMA prefetch
    # runs under batch b's compute (sim: -2 us; SBUF fits at ~183 KB/part)
    xpool = ctx.enter_context(
        tc.tile_pool(name="xp", bufs=int(os.environ.get("KXB", "17")))
    )
    popool = ctx.enter_context(tc.tile_pool(name="po", bufs=2))
    vapool = ctx.enter_context(tc.tile_pool(name="va", bufs=16))
    prpool = ctx.enter_context(
        tc.tile_pool(name="pr", bufs=int(os.environ.get("KPR", "4")))
    )
    nmpool = ctx.enter_context(tc.tile_pool(name="nm", bufs=9))
    rcpool = ctx.enter_context(tc.tile_pool(name="rc", bufs=2))
    oupool = ctx.enter_context(tc.tile_pool(name="ou", bufs=2))
    ps_mm = ctx.enter_context(tc.tile_pool(name="psmm", bufs=2, space="PSUM"))
    ps_sc = ctx.enter_context(tc.tile_pool(name="pssc", bufs=2, space="PSUM"))
    ps_cx = ctx.enter_context(tc.tile_pool(name="pscx", bufs=2, space="PSUM"))

    # constants: weights (fp32r, [128, KC*128] with k-chunk c at cols 128c),
    # biases [128, 1], identity [128, 64], packed v-weights [128, KC*VW]
    wt, bt = {}, {}

    def load_consts(names):
        for p in names:
            i = PROJ4.index(p)
            wt[p] = const.tile([128, KC * 128], MM, tag=f"w_{p}", name=f"w_{p}")
            nc.gpsimd.dma_start(
                out=wt[p].rearrange("p (c n) -> p c n", n=128),
                in_=prm["ipack"][:, WOFF + CW * i : WOFF + CW * (i + 1)].rearrange(
                    "(c p) n -> p c n", p=128
                ),
            )
            bt[p] = const.tile([128, 1], f32, tag=f"b_{p}", name=f"b_{p}")
            nc.gpsimd.dma_start(
                out=bt[p][:], in_=prm["ipack"][0:CW, BOFF + i : BOFF + i + 1]
            )

    load_consts(["q", "k"])
    ident = const.tile([128, DH], f32, tag="ident", name="ident")
    nc.gpsimd.dma_start(out=ident[:], in_=prm["ipack"][0:128, IOFF : IOFF + DH])

    def load_vw():
        vw = const.tile([128, KC * VW], MM, tag="vw", name="vw")
        nc.gpsimd.dma_start(
            out=vw.rearrange("p (c n) -> p c n", n=VW),
            in_=prm["ipack"][:, VOFF : VOFF + VW].rearrange("(c p) n -> p c n", p=128),
        )
        vwb = const.tile([1, VW], MM, tag="vwb", name="vwb")
        nc.gpsimd.dma_start(out=vwb[:], in_=prm["ipack"][0:1, VBOFF : VBOFF + VW])
        ones = const.tile([1, 128], MM, tag="ones", name="ones")
        nc.gpsimd.dma_start(out=ones[:], in_=prm["ipack"][0:1, OOFF : OOFF + 128])
        return vw, vwb, ones

    vw = vwb = ones = None

    # per-batch state handed from proj gen to attn gen
    projT = [None] * B   # dict p -> [128, S] tile (qT/kT/sqT/skT fp32r)
    vaug = [None] * B    # list per jc -> [128, VW] fp32r tile (v/sv natural+ones)

    # KXQ=1 (experimental, default off: the bitcast DMA fails neuronxcc
    # walrus codegen): x loads on the SP DMA queue so they don't queue
    # behind the weight loads on the gpsimd queue. SP DMAs cannot cast, so
    # that path needs a bitcast f32 view of the f32r tile.
    xq_sync = os.environ.get("KXQ", "0") == "1"
    xq_engine = nc.sync if xq_sync else nc.gpsimd

    # KXONCE=1 (timing diagnostic only — wrong data on reps >= 2): emit the
    # x-chunk DMAs only on the first KREP rep, so the KREP slope measures
    # per-iteration time WITHOUT the 16 MB/rep x reload. Comparing slopes
    # against the default isolates the x-DMA contribution to iteration time.
    xonce = os.environ.get("KXONCE", "0") == "1"

    def gen_proj(b, first=True):
        xt = []
        for kc in range(KC):
            t = xpool.tile([128, S], MM, tag="xt", name="xt")
            if first or not xonce:
                xq_engine.dma_start(
                    out=t.bitcast(f32) if (xq_sync and MM is not f32) else t[:],
                    in_=prm["ipack"][128 * kc : 128 * (kc + 1), S * b : S * (b + 1)],
                )
            xt.append(t)
        pj = {}
        for p in PROJ4:
            pj[p] = popool.tile([128, S], MM, tag=f"pj_{p}", name=f"pj_{p}")
        projT[b] = pj
        for tb in range(NTB):
            for pair in (("q", "k"), ("sq", "sk")):
                ps = {p: ps_mm.tile([128, 512], f32, tag="pmm", name=f"ps_{p}") for p in pair}
                for kc in range(KC):
                    for p in pair:
                        nc.tensor.matmul(
                            ps[p][:],
                            wt[p][:, 128 * kc : 128 * (kc + 1)],
                            xt[kc][:, 512 * tb : 512 * (tb + 1)],
                            start=(kc == 0),
                            stop=(kc == KC - 1),
                        )
                    yield
                for p in pair:
                    nc.vector.tensor_scalar_add(
                        pj[p][:, 512 * tb : 512 * (tb + 1)], ps[p][:], bt[p][:]
                    )
        # v/sv natural-layout projection with fused biases + ones columns
        va = []
        vaug[b] = va
        for blk in range(JC):
            tp = ps_mm.tile([128, VW], f32, tag="pmm", name="vn")
            for kc in range(KC):
                nc.tensor.matmul(
                    tp[:],
                    xt[kc][:, 128 * blk : 128 * (blk + 1)],
                    vw[:, VW * kc : VW * (kc + 1)],
                    start=(kc == 0),
                    stop=False,
                )
            nc.tensor.matmul(tp[:], ones[:], vwb[:], start=False, stop=True)
            yield
            t = vapool.tile([128, VW], MM, tag="vaug", name="vaug")
            va.append(t)
            nc.vector.tensor_copy(t[:], tp[:])
            yield

    def gen_attn(b):
        pj = projT[b]
        va = vaug[b]
        normed = {}
        for br in range(2):
            qT = pj["q" if br == 0 else "sq"]
            kT = pj["k" if br == 0 else "sk"]
            for ic in range(NIC):
                cx = {h: ps_cx.tile([128, 512], f32, tag="ctx", name=f"cx{h}") for h in range(HPC)}
                for jp in range(JC // 2):
                    pr = {}
                    for h in range(HPC):
                        sc = ps_sc.tile([128, 1024], f32, tag="sc", name="sc")
                        for half in range(2):
                            jc = 2 * jp + half
                            nc.tensor.matmul(
                                sc[:, 512 * half : 512 * (half + 1)],
                                kT[DH * h : DH * (h + 1), 128 * jc : 128 * (jc + 1)],
                                qT[DH * h : DH * (h + 1), 512 * ic : 512 * (ic + 1)],
                                start=True,
                                stop=True,
                            )
                        p = prpool.tile([128, 1024], MM, tag="probs", name="probs")
                        nc.scalar.activation(p[:], sc[:], Act.Exp)
                        pr[h] = p
                    yield
                    for h in range(HPC):
                        for half in range(2):
                            jc = 2 * jp + half
                            off = (DH + 1) * (2 * br + h)
                            nc.tensor.matmul(
                                cx[h][0 : DH + 1, :],
                                va[jc][:, off : off + DH + 1],
                                pr[h][:, 512 * half : 512 * (half + 1)],
                                start=(jc == 0),
                                stop=(jc == JC - 1),
                            )
                    yield
                for h in range(HPC):
                    if os.environ.get("KCXE", "0") == "1":
                        # Evacuate ctx PSUM->SBUF on the scalar engine (idle,
                        # and closer to PSUM) so the PSUM bank frees after one
                        # ~0.6us copy instead of after the whole
                        # reciprocal/broadcast/multiply chain — the next
                        # i-chunk's PV matmuls recycle the bank sooner.
                        ev = nmpool.tile([DH + 1, 512], dt.float32, tag="cxe", name="cxe")
                        nc.scalar.copy(ev[:], cx[h][0 : DH + 1, :])
                        src = ev
                    else:
                        src = cx[h]
                    rcp = rcpool.tile([1, 512], dt.float32, tag="rcp", name="rcp")
                    nc.vector.reciprocal(rcp[:], src[DH : DH + 1, :])
                    rcpb = rcpool.tile([DH, 512], dt.float32, tag="rcpb", name="rcpb")
                    nc.gpsimd.partition_broadcast(rcpb[:], rcp[:])
                    nt = nmpool.tile([DH, 512], dt.float32, tag="normed", name="normed")
                    nc.vector.tensor_tensor(nt[:], src[0:DH, :], rcpb[:], Alu.mult)
                    normed[br, h, ic] = nt
        outsb = oupool.tile([128, 8 * CW], dt.float32, tag="outsb", name="outsb")
        ov = outsb.rearrange("p (c w) -> p c w", w=CW)
        for h in range(HPC):
            tp = ps_mm.tile([128, 512], dt.float32, tag="pmm", name="otp")
            for ic in range(NIC):
                cb = nmpool.tile([DH, 512], dt.float32, tag="comb", name="comb", bufs=3)
                nc.vector.tensor_add(cb[:], normed[0, h, ic][:], normed[1, h, ic][:])
                for icc in range(4):
                    nc.tensor.transpose(
                        tp[:, DH * (4 * ic + icc) : DH * (4 * ic + icc + 1)],
                        cb[:, 128 * icc : 128 * (icc + 1)],
                        ident[0:DH, :],
                    )
            nc.vector.tensor_copy(
                ov[:, :, DH * h : DH * (h + 1)],
                tp.rearrange("p (c w) -> p c w", w=DH),
            )
            yield
        nc.sync.dma_start(
            out=prm["out"][b].rearrange("(c p) d -> p c d", p=128),
            in_=outsb.rearrange("p (c d) -> p c d", d=CW)
        )

    # driver: software-pipeline proj(b+1) into attention(b)'s rounds so the
    # PE always has dense matmul work while ACT chews through the exps.
    # KREP repeats the whole pipeline in-NEFF (timing: slope vs rep count).
    first_rep = True
    for rep in range(int(os.environ.get("KREP", "1"))):
        pgens = [gen_proj(b, rep == 0) for b in range(B)]
        next(pgens[0], None)
        if first_rep:
            load_consts(["sq", "sk"])
            vw, vwb, ones = load_vw()
            first_rep = False
        for _ in pgens[0]:
            pass
        for b in range(B):
            pg = pgens[b + 1] if b + 1 < B else None
            for _ in gen_attn(b):
                if pg is not None:
                    for _ in range(int(os.environ.get("KPULL", "2"))):
                        if next(pg, "done") == "done":
                            pg = None
                            break
            while pg is not None and next(pg, "done") != "done":
                pass


def build_nc():
    nc = bacc.Bacc("TRN2", target_bir_lowering=False, debug=False)
    prm = {}
    prm["ipack"] = nc.declare_dram_parameter(
        "ipack", [D, IPACK_COLS], dt.float32, isOutput=False
    )
    prm["out"] = nc.declare_dram_parameter("out", [B, S, CW], dt.float32, isOutput=True)

    from contextlib import ExitStack

    with tile.TileContext(nc) as tc:
        with ExitStack() as ctx:
            _emit(nc, tc, ctx, prm)
    nc.compile()
    return nc


def make_in_maps(hidden_states, Wq, bq, Wk, bk, Wv, bv, Wsq, bsq, Wsk, bsk, Wsv, bsv, attn_w):
    """Host-side sharding: slice per-head weight columns, fold scales, pack."""
    f32 = np.float32
    x = np.asarray(hidden_states, f32).reshape(B * S, D)
    xT = np.ascontiguousarray(x.T)
    a = np.asarray(attn_w, f32)
    e = np.exp(a - a.max())
    w = (e / e.sum()).astype(f32)
    sc = f32(1.0 / np.sqrt(DH))
    ident = np.tile(np.eye(DH, dtype=f32), (2, 1))

    full4 = {
        "q": (np.asarray(Wq, f32) * sc, np.asarray(bq, f32) * sc),
        "k": (np.asarray(Wk, f32), np.asarray(bk, f32)),
        "sq": (np.asarray(Wsq, f32) * sc, np.asarray(bsq, f32) * sc),
        "sk": (np.asarray(Wsk, f32), np.asarray(bsk, f32)),
    }
    Wv_f = np.asarray(Wv, f32) * w[0]
    bv_f = np.asarray(bv, f32) * w[0]
    Wsv_f = np.asarray(Wsv, f32) * w[1]
    bsv_f = np.asarray(bsv, f32) * w[1]

    in_maps = []
    for c in range(NCORES):
        cols = slice(CW * c, CW * (c + 1))
        ipack = np.zeros((D, IPACK_COLS), f32)
        ipack[:, XOFF : XOFF + B * S] = xT
        for i, p in enumerate(PROJ4):
            W, b = full4[p]
            ipack[:, WOFF + CW * i : WOFF + CW * (i + 1)] = W[:, cols]
            ipack[0:CW, BOFF + i] = b[cols]
        ipack[0:128, IOFF : IOFF + DH] = ident
        # vw: [v_h0 | 1s | v_h1 | 1s | sv_h0 | 1s | sv_h1 | 1s] cols, with the
        # ones columns zero in the weight rows; bias row supplies bias + 1.
        for hb, (Wm, bm) in enumerate(
            [(Wv_f[:, cols], bv_f[cols]), (Wsv_f[:, cols], bsv_f[cols])]
        ):
            for h in range(HPC):
                off = VOFF + (DH + 1) * (HPC * hb + h)
                ipack[:, off : off + DH] = Wm[:, DH * h : DH * (h + 1)]
                boff = VBOFF + (DH + 1) * (HPC * hb + h)
                ipack[0, boff : boff + DH] = bm[DH * h : DH * (h + 1)]
                ipack[0, boff + DH] = 1.0
        ipack[0, OOFF : OOFF + 128] = 1.0
        in_maps.append({"ipack": ipack})
    return in_maps


_NC_CACHE = {}


def get_nc():
    if "nc" not in _NC_CACHE:
        _NC_CACHE["nc"] = build_nc()
    return _NC_CACHE["nc"]


def kernel(**inputs):
    nc = get_nc()
    in_maps = make_in_maps(**inputs)
    res = run_bass_kernel_spmd(nc, in_maps, list(range(NCORES)))
    parts = [res.results[c]["out"] for c in range(NCORES)]
    return np.concatenate(parts, axis=2).astype(np.float32)

